# revision 11
# baseline (speedup 1.0000x reference)
"""Trainium2 Bass kernel for nn_AnisotropicDilatedProjectM2.

Op: out[b,c,y,x] = max_{o,dy,dx} ( x[b,c,o,y+dy,x+dx] - cost[o,dy,dx] )
with cost an anisotropic elliptical HJB dilation kernel (+inf outside the
ellipse), 11x11 window, Or=8 orientations, max-reduced over orientation.

Sharding: data-parallel over batch B=8 -> 8 NeuronCores, zero comm.

Raw-bass implementation (no Tile): one DVE program does all the fused
shifted-subtract-max ops; the SP program streams per-orientation slabs
into ping-pong SBUF buffers one orientation ahead.  Layout: partition
p = 16*b + c (row-block b, channel c); each partition holds 42 rows
(32 output rows + 2*5 halo) of its (c, o) image; (dy,dx) shifts are
free-axis APs, with out-of-image columns handled by shrinking the AP
and out-of-image rows by -1e30 pad rows memset once at start.
"""

import os
import sys
import numpy as np
from math import ceil, pi

if os.path.isdir("/opt/trn_rl_repo"):
    sys.path.insert(0, "/opt/trn_rl_repo")

import concourse.bass as bass
from concourse import mybir
from concourse.bass_utils import run_bass_kernel_spmd

B, C, Or, H, W = 8, 16, 8, 256, 256
LONGITUDINAL, LATERAL, ALPHA, T = 5.0, 2.5, 2.0 / 3.0, 1.0
R = int(ceil(max(LONGITUDINAL, LATERAL)))  # 5
K = 2 * R + 1  # 11
BLK = H // 8  # 32 rows per partition block
HROWS = BLK + 2 * R  # 42
NEG = -1.0e30
F32 = mybir.dt.float32


def _make_cost() -> np.ndarray:
    """Replicates reference._make_cost exactly (float64 -> float32)."""
    offs = np.arange(-R, R + 1, dtype=np.float64)
    dy, dx = np.meshgrid(offs, offs, indexing="ij")
    thetas = np.arange(Or, dtype=np.float64) * (pi / Or)
    ct = np.cos(thetas)[:, None, None]
    st = np.sin(thetas)[:, None, None]
    lon = ct * dx[None] + st * dy[None]
    lat = -st * dx[None] + ct * dy[None]
    rho2 = (lon / LONGITUDINAL) ** 2 + (lat / LATERAL) ** 2
    p = (2.0 * ALPHA) / (2.0 * ALPHA - 1.0)
    coef = (2.0 * ALPHA - 1.0) / (2.0 * ALPHA)
    cost = T * coef * np.power(rho2, p / 2.0) / (T**p)
    cost = np.where(rho2 <= 1.0, cost, np.inf).astype(np.float32)
    return cost  # [Or, K, K]; index [o, dy+R, dx+R]


def _candidates():
    """Per-orientation list of (dy, dx, cost) with finite cost."""
    cost = _make_cost()
    cands = []
    for o in range(Or):
        lst = []
        for iy in range(K):
            for ix in range(K):
                c = cost[o, iy, ix]
                if np.isfinite(c):
                    lst.append((iy - R, ix - R, float(c)))
        cands.append(lst)
    return cands


def _build_nc():
    cands = _candidates()
    nc = bass.Bass()
    x_ext = nc.declare_dram_parameter("x", [C, Or, H, W], F32, isOutput=False)
    out_ext = nc.declare_dram_parameter("out", [C, H, W], F32, isOutput=True)

    from contextlib import ExitStack

    with ExitStack() as ctx:
        block = ctx.enter_context(nc.Block())
        # One load-completion semaphore per slab parity: stages 0 and 1 are
        # both in flight initially, so a single counter could reach the
        # stage-0 threshold with stage-1 completions.
        dma_e = ctx.enter_context(nc.semaphore("dma_e"))
        dma_o = ctx.enter_context(nc.semaphore("dma_o"))
        out_sem = ctx.enter_context(nc.semaphore("out_sem"))
        cmp_sem = ctx.enter_context(nc.semaphore("cmp_sem"))
        init_sem = ctx.enter_context(nc.semaphore("init_sem"))
        S0 = ctx.enter_context(nc.sbuf_tensor("slab0", [128, HROWS, W], F32))
        S1 = ctx.enter_context(nc.sbuf_tensor("slab1", [128, HROWS, W], F32))
        acc = ctx.enter_context(nc.sbuf_tensor("acc", [128, BLK, W], F32))
        slabs = [S0, S1]

        @block.sync
        def _(sp: bass.BassEngine):
            sp.wait_ge(init_sem, 1)
            for o in range(Or):
                if o >= 2:
                    # slab (o-2)'s readers (stage o-2 STTs) must be done
                    sp.wait_ge(cmp_sem, o - 1)
                S = slabs[o % 2]
                sem = dma_e if o % 2 == 0 else dma_o
                # blocks 1..6 in one overlapping-AP DMA: partition dims
                # (b outer x6, c inner x16), rows 42 starting at 32b-5.
                src_mid = bass.AP(
                    x_ext,
                    o * H * W + (BLK - R) * W,
                    [[BLK * W, 6], [Or * H * W, 16], [W, HROWS], [1, W]],
                )
                sp.dma_start(out=S[16:112, :, :], in_=src_mid).then_inc(sem, 16)
                # block 0: rows 0..37 into slab rows 5..42
                sp.dma_start(
                    out=S[0:16, R:HROWS, :], in_=x_ext[:, o, 0 : BLK + R, :]
                ).then_inc(sem, 16)
                # block 7: rows 219..256 into slab rows 0..37
                sp.dma_start(
                    out=S[112:128, 0 : BLK + R, :],
                    in_=x_ext[:, o, H - BLK - R : H, :],
                ).then_inc(sem, 16)
            sp.wait_ge(cmp_sem, Or)
            # single gathered output DMA: partition p = 16b + c
            dst = bass.AP(
                out_ext,
                0,
                [[BLK * W, 8], [H * W, 16], [W, BLK], [1, W]],
            )
            sp.dma_start(out=dst, in_=acc[:, :, :]).then_inc(out_sem, 16)
            sp.wait_ge(out_sem, 16)

        @block.vector
        def _(ve: bass.BassVectorEngine):
            ve.memset(acc[:, :, :], NEG)
            # -inf pad rows: top halo rows of block 0, bottom halo rows of
            # block 7.  Engine partition ranges must be 32-aligned, so the
            # memsets also cover blocks 1 resp. 6, whose rows are rewritten
            # by every stage DMA before being read.
            for S in slabs:
                ve.memset(S[0:32, 0:R, :], NEG)
                ve.memset(S[96:128, HROWS - R : HROWS, :], NEG)
            ve.memset(acc[0:32, 0:1, 0:1], NEG).then_inc(init_sem, 1)
            for o in range(Or):
                ve.wait_ge(dma_e if o % 2 == 0 else dma_o, 16 * 3 * (o // 2 + 1))
                S = slabs[o % 2]
                last = None
                for dy, dx, c in cands[o]:
                    x0, x1 = max(0, -dx), min(W, W - dx)
                    last = ve.scalar_tensor_tensor(
                        out=acc[:, :, x0:x1],
                        in0=S[:, dy + R : dy + R + BLK, x0 + dx : x1 + dx],
                        scalar=c,
                        in1=acc[:, :, x0:x1],
                        op0=mybir.AluOpType.subtract,
                        op1=mybir.AluOpType.max,
                    )
                last.then_inc(cmp_sem, 1)

    return nc


_NC_CACHE = None


def _get_nc():
    global _NC_CACHE
    if _NC_CACHE is None:
        _NC_CACHE = _build_nc()
    return _NC_CACHE


def kernel(**inputs) -> np.ndarray:
    x = np.asarray(inputs["x"], dtype=np.float32)
    assert x.shape == (B, C, Or, H, W), x.shape
    nc = _get_nc()
    in_maps = [{"x": np.ascontiguousarray(x[i])} for i in range(B)]
    trace = bool(int(os.environ.get("BASS_KERNEL_TRACE", "0")))
    res = run_bass_kernel_spmd(nc, in_maps, core_ids=list(range(B)), trace=trace)
    if trace:
        kernel.last_exec_time_ns = res.exec_time_ns
        kernel.last_results = res
    out = np.stack([res.results[i]["out"] for i in range(B)], axis=0)
    return out.astype(np.float32, copy=False)


# revision 15
# speedup vs baseline: 1.2769x; 1.2769x over previous
"""Trainium2 Bass kernel for nn_AnisotropicDilatedProjectM2.

Op: out[b,c,y,x] = max_{o,dy,dx} ( x[b,c,o,y+dy,x+dx] - cost[o,dy,dx] )
with cost an anisotropic elliptical HJB dilation kernel (+inf outside the
ellipse), 11x11 window, Or=8 orientations, max-reduced over orientation.

Sharding: data-parallel over batch B=8 -> 8 NeuronCores, zero comm.

Raw-bass multi-engine implementation:
  SP   : streams one f32 slab per orientation (3 DMAs incl. an
         overlapping-AP load for interior row-blocks), output DMA.
  ACT  : converts the f32 slab to a bf16 slab E (picking up -1e30 row
         pads), plus a 1-element-shifted copy O (for 4B alignment so
         DVE tensor_tensor runs in its 2x bf16 mode for either shift
         parity).
  DVE  : main accumulator.  Candidates are processed in +-delta pairs
         (cost is centrally symmetric): tmp = max(E[+d], E[-d]);
         tmp -= cost (tensor_scalar 4x); acc = max(acc, tmp) (2x).
  POOL : GPSIMD takes a share of pairs as two fused scalar_tensor_tensor
         singles into its own accumulator acc2; merged at the end.

Layout: partition p = 16*b + c (row-block b, channel c); each partition
holds 42 rows (32 output + 2*5 halo) x 266 cols (256 + 2*5 -1e30 col
pads) of its (c, o) image; all (dy,dx) shifts are free-axis APs.
"""

import os
import sys
import numpy as np
from math import ceil, pi

if os.path.isdir("/opt/trn_rl_repo"):
    sys.path.insert(0, "/opt/trn_rl_repo")

import concourse.bass as bass
from concourse import mybir
from concourse.bass_utils import run_bass_kernel_spmd

B, C, Or, H, W = 8, 16, 8, 256, 256
LONGITUDINAL, LATERAL, ALPHA, T = 5.0, 2.5, 2.0 / 3.0, 1.0
R = int(ceil(max(LONGITUDINAL, LATERAL)))  # 5
K = 2 * R + 1  # 11
BLK = H // 8  # 32 rows per partition block
HROWS = BLK + 2 * R  # 42
PW = W + 2 * R  # 266 padded width
NEG = -1.0e30
F32 = mybir.dt.float32
BF16 = mybir.dt.bfloat16

# engine time constants (ns) for static load balancing
DVE_PAIR_NS = (4247 + 2106 + 4247) / 0.96  # TT2x + TSP4x + TT2x on 8192 elems
GPS_PAIR_NS = 2 * 8192 * (1.0 / 1.2) / 0.60  # two 1x STT singles at 0.6 eff


def _make_cost() -> np.ndarray:
    """Replicates reference._make_cost exactly (float64 -> float32)."""
    offs = np.arange(-R, R + 1, dtype=np.float64)
    dy, dx = np.meshgrid(offs, offs, indexing="ij")
    thetas = np.arange(Or, dtype=np.float64) * (pi / Or)
    ct = np.cos(thetas)[:, None, None]
    st = np.sin(thetas)[:, None, None]
    lon = ct * dx[None] + st * dy[None]
    lat = -st * dx[None] + ct * dy[None]
    rho2 = (lon / LONGITUDINAL) ** 2 + (lat / LATERAL) ** 2
    p = (2.0 * ALPHA) / (2.0 * ALPHA - 1.0)
    coef = (2.0 * ALPHA - 1.0) / (2.0 * ALPHA)
    cost = T * coef * np.power(rho2, p / 2.0) / (T**p)
    cost = np.where(rho2 <= 1.0, cost, np.inf).astype(np.float32)
    return cost  # [Or, K, K]; index [o, dy+R, dx+R]


def _schedule():
    """Per-orientation: list of pairs [(dy,dx,cost)] (dy,dx) the positive
    representative, and split into DVE pairs vs GPSIMD pairs so both
    engines finish together.  The center (0,0,0.0) rides as a half-pair."""
    cost = _make_cost()
    per_o = []
    for o in range(Or):
        pairs = []
        center = None
        for iy in range(K):
            for ix in range(K):
                c = cost[o, iy, ix]
                if not np.isfinite(c):
                    continue
                dy, dx = iy - R, ix - R
                if (dy, dx) == (0, 0):
                    center = float(c)
                    continue
                if (dy, dx) > (-dy, -dx):
                    continue  # keep one representative per +-pair
                pairs.append((dy, dx, float(c)))
        per_o.append((pairs, center))

    # global greedy split: GPSIMD takes pairs (2 singles each) until its
    # projected time would exceed DVE's remaining time.
    total_pairs = sum(len(p) for p, _ in per_o)
    n_gps = 0
    if int(os.environ.get("GPS_PAIRS", "-1")) >= 0:
        n_gps = int(os.environ["GPS_PAIRS"])
    else:
        while (n_gps + 1) * GPS_PAIR_NS <= (total_pairs - (n_gps + 1)) * DVE_PAIR_NS:
            n_gps += 1
    # distribute gps pairs evenly across orientations
    sched = []
    gps_left, pairs_left = n_gps, total_pairs
    for o in range(Or):
        pairs, center = per_o[o]
        k = round(gps_left * len(pairs) / max(pairs_left, 1))
        k = min(k, len(pairs), gps_left)
        # give GPSIMD the pairs with even dx (they'd be unaligned/1x on DVE)
        pairs_sorted = sorted(pairs, key=lambda t: (t[1] % 2 != 0))
        gps_pairs = pairs_sorted[:k]
        dve_pairs = pairs_sorted[k:]
        gps_left -= k
        pairs_left -= len(pairs)
        sched.append((dve_pairs, gps_pairs, center))
    return sched


def _build_nc():
    sched = _schedule()
    HAS_GPS = any(len(g) for _, g, _ in sched)
    nc = bass.Bass()
    x_ext = nc.declare_dram_parameter("x", [C, Or, H, W], F32, isOutput=False)
    out_ext = nc.declare_dram_parameter("out", [C, H, W], F32, isOutput=True)

    from contextlib import ExitStack

    with ExitStack() as ctx:
        block = ctx.enter_context(nc.Block())
        initD = ctx.enter_context(nc.semaphore("initD"))
        dmaS = ctx.enter_context(nc.semaphore("dmaS"))
        convA = ctx.enter_context(nc.semaphore("convA"))
        cmpD = ctx.enter_context(nc.semaphore("cmpD"))
        cmpG = ctx.enter_context(nc.semaphore("cmpG"))
        mrgD = ctx.enter_context(nc.semaphore("mrgD"))
        out_sem = ctx.enter_context(nc.semaphore("out_sem"))
        Sf = ctx.enter_context(nc.sbuf_tensor("slab_f32", [128, HROWS, W], F32))
        E0 = ctx.enter_context(nc.sbuf_tensor("E0", [128, HROWS, PW], BF16))
        E1 = ctx.enter_context(nc.sbuf_tensor("E1", [128, HROWS, PW], BF16))
        O0 = ctx.enter_context(nc.sbuf_tensor("O0", [128, HROWS, PW], BF16))
        O1 = ctx.enter_context(nc.sbuf_tensor("O1", [128, HROWS, PW], BF16))
        acc = ctx.enter_context(nc.sbuf_tensor("acc", [128, BLK, W], BF16))
        acc2 = ctx.enter_context(nc.sbuf_tensor("acc2", [128, BLK, W], BF16))
        tmp = ctx.enter_context(nc.sbuf_tensor("tmp", [128, BLK, W], BF16))
        Es, Os = [E0, E1], [O0, O1]

        def eo_ap(o, dy, dx):
            """Operand AP for shift (dy,dx) on stage-o slab, routed through
            E (even flat offset) or O (odd) so the innermost AP start is
            4-byte aligned -> DVE 2x bf16 mode."""
            f_col = R + dx  # column of first element within the padded row
            row0 = dy + R
            if f_col % 2 == 0:  # (row0*PW + f_col) even since PW even
                return Es[o % 2][:, row0 : row0 + BLK, f_col : f_col + W]
            return Os[o % 2][:, row0 : row0 + BLK, f_col - 1 : f_col - 1 + W]

        @block.sync
        def _(sp: bass.BassEngine):
            sp.wait_ge(initD, 1)
            for o in range(Or):
                if o >= 1:
                    sp.wait_ge(convA, o)  # Sf free (ACT conv o-1 done)
                # blocks 1..6 via one overlapping-AP DMA (p = 16b + c)
                src_mid = bass.AP(
                    x_ext,
                    o * H * W + (BLK - R) * W,
                    [[BLK * W, 6], [Or * H * W, 16], [W, HROWS], [1, W]],
                )
                sp.dma_start(out=Sf[16:112, :, :], in_=src_mid).then_inc(dmaS, 16)
                sp.dma_start(
                    out=Sf[0:16, R:HROWS, :], in_=x_ext[:, o, 0 : BLK + R, :]
                ).then_inc(dmaS, 16)
                sp.dma_start(
                    out=Sf[112:128, 0 : BLK + R, :],
                    in_=x_ext[:, o, H - BLK - R : H, :],
                ).then_inc(dmaS, 16)
            sp.wait_ge(mrgD, 1)
            dst = bass.AP(out_ext, 0, [[BLK * W, 8], [H * W, 16], [W, BLK], [1, W]])
            sp.dma_start(out=dst, in_=Sf[:, 0:BLK, :]).then_inc(out_sem, 16)
            sp.wait_ge(out_sem, 16)

        @block.scalar
        def _(act: bass.BassScalarEngine):
            for o in range(Or):
                act.wait_ge(dmaS, 16 * 3 * (o + 1))  # slab o loaded
                if o >= 2:
                    # E/O[o%2] still being read by stage o-2 consumers
                    act.wait_ge(cmpD, o - 1)
                    if HAS_GPS:
                        act.wait_ge(cmpG, o - 1)
                E, O = Es[o % 2], Os[o % 2]
                # f32 -> bf16 convert into padded interior (pad rows of Sf
                # are -1e30 and pass through, refreshing E's row pads)
                act.copy(E[:, :, R : R + W], Sf[:, :, :])
                # O = E shifted left by one column
                act.copy(O[:, :, 0 : PW - 1], E[:, :, 1:PW]).then_inc(convA, 1)

        @block.vector
        def _(ve: bass.BassVectorEngine):
            ve.memset(acc[:, :, :], NEG)
            for S in (E0, E1, O0, O1):
                ve.memset(S[:, :, 0:R], NEG)
                ve.memset(S[:, :, R + W :], NEG)
            # -1e30 pad rows of the f32 slab (blocks 0 / 7 halo; engine
            # partition base must be 32-aligned, blocks 1/6 are re-DMAed)
            ve.memset(Sf[0:32, 0:R, :], NEG)
            ve.memset(Sf[96:128, HROWS - R : HROWS, :], NEG)
            ve.memset(acc[0:32, 0:1, 0:1], NEG).then_inc(initD, 1)
            for o in range(Or):
                ve.wait_ge(convA, o + 1)
                dve_pairs, _, center = sched[o]
                last = None
                if center is not None:
                    # cost at the center is exactly 0.0: plain max
                    last = ve.tensor_tensor(
                        out=acc[:, :, :],
                        in0=acc[:, :, :],
                        in1=eo_ap(o, 0, 0),
                        op=mybir.AluOpType.max,
                    )
                for dy, dx, c in dve_pairs:
                    ve.tensor_tensor(
                        out=tmp[:, :, :],
                        in0=eo_ap(o, dy, dx),
                        in1=eo_ap(o, -dy, -dx),
                        op=mybir.AluOpType.max,
                    )
                    ve.tensor_scalar_sub(tmp[:, :, :], tmp[:, :, :], c)
                    last = ve.tensor_tensor(
                        out=acc[:, :, :],
                        in0=acc[:, :, :],
                        in1=tmp[:, :, :],
                        op=mybir.AluOpType.max,
                    )
                last.then_inc(cmpD, 1)
            # merge GPSIMD accumulator and emit f32 into Sf's first 32 rows
            if HAS_GPS:
                ve.wait_ge(cmpG, Or)
                ve.tensor_tensor(
                    out=acc[:, :, :],
                    in0=acc[:, :, :],
                    in1=acc2[:, :, :],
                    op=mybir.AluOpType.max,
                )
            ve.tensor_copy(Sf[:, 0:BLK, :], acc[:, :, :]).then_inc(mrgD, 1)

        if HAS_GPS:

            @block.gpsimd
            def _(gp: bass.BassGpSimd):
                gp.memset(acc2[:, :, :], NEG)
                for o in range(Or):
                    gp.wait_ge(convA, o + 1)
                    _, gps_pairs, _ = sched[o]
                    last = None
                    for dy, dx, c in gps_pairs:
                        last = gp.tensor_tensor(
                            out=acc2[:, :, :],
                            in0=eo_ap(o, dy, dx),
                            in1=eo_ap(o, -dy, -dx),
                            op=mybir.AluOpType.max,
                        )
                    last.then_inc(cmpG, 1)

    return nc


_NC_CACHE = None


def _get_nc():
    global _NC_CACHE
    if _NC_CACHE is None:
        _NC_CACHE = _build_nc()
    return _NC_CACHE


def kernel(**inputs) -> np.ndarray:
    x = np.asarray(inputs["x"], dtype=np.float32)
    assert x.shape == (B, C, Or, H, W), x.shape
    nc = _get_nc()
    in_maps = [{"x": np.ascontiguousarray(x[i])} for i in range(B)]
    trace = bool(int(os.environ.get("BASS_KERNEL_TRACE", "0")))
    res = run_bass_kernel_spmd(nc, in_maps, core_ids=list(range(B)), trace=trace)
    if trace:
        kernel.last_exec_time_ns = res.exec_time_ns
        kernel.last_results = res
    out = np.stack([res.results[i]["out"] for i in range(B)], axis=0)
    return out.astype(np.float32, copy=False)


# revision 16
# speedup vs baseline: 1.7007x; 1.3319x over previous
"""Trainium2 Bass kernel for nn_AnisotropicDilatedProjectM2.

Op: out[b,c,y,x] = max_{o,dy,dx} ( x[b,c,o,y+dy,x+dx] - cost[o,dy,dx] )
with cost an anisotropic elliptical HJB dilation kernel (+inf outside the
ellipse), 11x11 window, Or=8 orientations, max-reduced over orientation.

Sharding: data-parallel over batch B=8 -> 8 NeuronCores, zero comm.

Raw-bass multi-engine implementation:
  SP   : streams one f32 slab per orientation (3 DMAs incl. an
         overlapping-AP load for interior row-blocks), output DMA.
  ACT  : converts the f32 slab to a bf16 slab E (picking up -1e30 row
         pads), plus a 1-element-shifted copy O (for 4B alignment so
         DVE tensor_tensor runs in its 2x bf16 mode for either shift
         parity).
  DVE  : main accumulator.  Candidates are processed in +-delta pairs
         (cost is centrally symmetric): tmp = max(E[+d], E[-d]);
         tmp -= cost (tensor_scalar 4x); acc = max(acc, tmp) (2x).
  POOL : GPSIMD takes a share of pairs as two fused scalar_tensor_tensor
         singles into its own accumulator acc2; merged at the end.

Layout: partition p = 16*b + c (row-block b, channel c); each partition
holds 42 rows (32 output + 2*5 halo) x 266 cols (256 + 2*5 -1e30 col
pads) of its (c, o) image; all (dy,dx) shifts are free-axis APs.
"""

import os
import sys
import numpy as np
from math import ceil, pi

if os.path.isdir("/opt/trn_rl_repo"):
    sys.path.insert(0, "/opt/trn_rl_repo")

import concourse.bass as bass
from concourse import mybir
from concourse.bass_utils import run_bass_kernel_spmd

B, C, Or, H, W = 8, 16, 8, 256, 256
LONGITUDINAL, LATERAL, ALPHA, T = 5.0, 2.5, 2.0 / 3.0, 1.0
R = int(ceil(max(LONGITUDINAL, LATERAL)))  # 5
K = 2 * R + 1  # 11
BLK = H // 8  # 32 rows per partition block
HROWS = BLK + 2 * R  # 42
PW = W + 2 * R  # 266 padded width
NEG = -1.0e30
F32 = mybir.dt.float32
BF16 = mybir.dt.bfloat16

# engine time constants (ns) for static load balancing
DVE_PAIR_NS = (4247 + 2106 + 4247) / 0.96  # TT2x + TSP4x + TT2x on 8192 elems
GPS_PAIR_NS = 2 * 8192 * (1.0 / 1.2) / 0.60  # two 1x STT singles at 0.6 eff


def _make_cost() -> np.ndarray:
    """Replicates reference._make_cost exactly (float64 -> float32)."""
    offs = np.arange(-R, R + 1, dtype=np.float64)
    dy, dx = np.meshgrid(offs, offs, indexing="ij")
    thetas = np.arange(Or, dtype=np.float64) * (pi / Or)
    ct = np.cos(thetas)[:, None, None]
    st = np.sin(thetas)[:, None, None]
    lon = ct * dx[None] + st * dy[None]
    lat = -st * dx[None] + ct * dy[None]
    rho2 = (lon / LONGITUDINAL) ** 2 + (lat / LATERAL) ** 2
    p = (2.0 * ALPHA) / (2.0 * ALPHA - 1.0)
    coef = (2.0 * ALPHA - 1.0) / (2.0 * ALPHA)
    cost = T * coef * np.power(rho2, p / 2.0) / (T**p)
    cost = np.where(rho2 <= 1.0, cost, np.inf).astype(np.float32)
    return cost  # [Or, K, K]; index [o, dy+R, dx+R]


def _schedule():
    """Per-orientation: list of pairs [(dy,dx,cost)] (dy,dx) the positive
    representative, and split into DVE pairs vs GPSIMD pairs so both
    engines finish together.  The center (0,0,0.0) rides as a half-pair."""
    cost = _make_cost()
    per_o = []
    for o in range(Or):
        pairs = []
        center = None
        for iy in range(K):
            for ix in range(K):
                c = cost[o, iy, ix]
                if not np.isfinite(c):
                    continue
                dy, dx = iy - R, ix - R
                if (dy, dx) == (0, 0):
                    center = float(c)
                    continue
                if (dy, dx) > (-dy, -dx):
                    continue  # keep one representative per +-pair
                pairs.append((dy, dx, float(c)))
        per_o.append((pairs, center))

    # global greedy split: GPSIMD takes pairs (2 singles each) until its
    # projected time would exceed DVE's remaining time.
    total_pairs = sum(len(p) for p, _ in per_o)
    n_gps = 0
    if int(os.environ.get("GPS_PAIRS", "-1")) >= 0:
        n_gps = int(os.environ["GPS_PAIRS"])
    else:
        while (n_gps + 1) * GPS_PAIR_NS <= (total_pairs - (n_gps + 1)) * DVE_PAIR_NS:
            n_gps += 1
    # distribute gps pairs evenly across orientations
    sched = []
    gps_left, pairs_left = n_gps, total_pairs
    band_eps = float(os.environ.get("BAND_EPS", "0.01"))
    for o in range(Or):
        pairs, center = per_o[o]
        k = round(gps_left * len(pairs) / max(pairs_left, 1))
        k = min(k, len(pairs), gps_left)
        # give GPSIMD the pairs with even dx (they'd be unaligned/1x on DVE)
        pairs_sorted = sorted(pairs, key=lambda t: (t[1] % 2 != 0))
        gps_pairs = pairs_sorted[:k]
        dve_pairs = pairs_sorted[k:]
        gps_left -= k
        pairs_left -= len(pairs)
        # band DVE pairs by cost: within a band (spread <= band_eps) all
        # members share one max-tree and a single subtract of the band mid
        bands = []
        for dy, dx, c in sorted(dve_pairs, key=lambda t: t[2]):
            if bands and c - bands[-1][0][2] <= band_eps:
                bands[-1].append((dy, dx, c))
            else:
                bands.append([(dy, dx, c)])
        sched.append((bands, gps_pairs, center))
    return sched


def _build_nc():
    sched = _schedule()
    HAS_GPS = any(len(g) for _, g, _ in sched)
    nc = bass.Bass()
    x_ext = nc.declare_dram_parameter("x", [C, Or, H, W], F32, isOutput=False)
    out_ext = nc.declare_dram_parameter("out", [C, H, W], F32, isOutput=True)

    from contextlib import ExitStack

    with ExitStack() as ctx:
        block = ctx.enter_context(nc.Block())
        initD = ctx.enter_context(nc.semaphore("initD"))
        dmaS = ctx.enter_context(nc.semaphore("dmaS"))
        convA = ctx.enter_context(nc.semaphore("convA"))
        cmpD = ctx.enter_context(nc.semaphore("cmpD"))
        cmpG = ctx.enter_context(nc.semaphore("cmpG"))
        mrgD = ctx.enter_context(nc.semaphore("mrgD"))
        out_sem = ctx.enter_context(nc.semaphore("out_sem"))
        Sf = ctx.enter_context(nc.sbuf_tensor("slab_f32", [128, HROWS, W], F32))
        E0 = ctx.enter_context(nc.sbuf_tensor("E0", [128, HROWS, PW], BF16))
        E1 = ctx.enter_context(nc.sbuf_tensor("E1", [128, HROWS, PW], BF16))
        O0 = ctx.enter_context(nc.sbuf_tensor("O0", [128, HROWS, PW], BF16))
        O1 = ctx.enter_context(nc.sbuf_tensor("O1", [128, HROWS, PW], BF16))
        acc = ctx.enter_context(nc.sbuf_tensor("acc", [128, BLK, W], BF16))
        acc2 = ctx.enter_context(nc.sbuf_tensor("acc2", [128, BLK, W], BF16))
        tmp = ctx.enter_context(nc.sbuf_tensor("tmp", [128, BLK, W], BF16))
        Es, Os = [E0, E1], [O0, O1]

        def eo_ap(o, dy, dx):
            """Operand AP for shift (dy,dx) on stage-o slab, routed through
            E (even flat offset) or O (odd) so the innermost AP start is
            4-byte aligned -> DVE 2x bf16 mode."""
            f_col = R + dx  # column of first element within the padded row
            row0 = dy + R
            if f_col % 2 == 0:  # (row0*PW + f_col) even since PW even
                return Es[o % 2][:, row0 : row0 + BLK, f_col : f_col + W]
            return Os[o % 2][:, row0 : row0 + BLK, f_col - 1 : f_col - 1 + W]

        @block.sync
        def _(sp: bass.BassEngine):
            sp.wait_ge(initD, 1)
            for o in range(Or):
                if o >= 1:
                    sp.wait_ge(convA, o)  # Sf free (ACT conv o-1 done)
                # blocks 1..6 via one overlapping-AP DMA (p = 16b + c)
                src_mid = bass.AP(
                    x_ext,
                    o * H * W + (BLK - R) * W,
                    [[BLK * W, 6], [Or * H * W, 16], [W, HROWS], [1, W]],
                )
                sp.dma_start(out=Sf[16:112, :, :], in_=src_mid).then_inc(dmaS, 16)
                sp.dma_start(
                    out=Sf[0:16, R:HROWS, :], in_=x_ext[:, o, 0 : BLK + R, :]
                ).then_inc(dmaS, 16)
                sp.dma_start(
                    out=Sf[112:128, 0 : BLK + R, :],
                    in_=x_ext[:, o, H - BLK - R : H, :],
                ).then_inc(dmaS, 16)
            sp.wait_ge(mrgD, 1)
            dst = bass.AP(out_ext, 0, [[BLK * W, 8], [H * W, 16], [W, BLK], [1, W]])
            sp.dma_start(out=dst, in_=Sf[:, 0:BLK, :]).then_inc(out_sem, 16)
            sp.wait_ge(out_sem, 16)

        @block.scalar
        def _(act: bass.BassScalarEngine):
            for o in range(Or):
                act.wait_ge(dmaS, 16 * 3 * (o + 1))  # slab o loaded
                if o >= 2:
                    # E/O[o%2] still being read by stage o-2 consumers
                    act.wait_ge(cmpD, o - 1)
                    if HAS_GPS:
                        act.wait_ge(cmpG, o - 1)
                E, O = Es[o % 2], Os[o % 2]
                # f32 -> bf16 convert into padded interior (pad rows of Sf
                # are -1e30 and pass through, refreshing E's row pads)
                act.copy(E[:, :, R : R + W], Sf[:, :, :])
                # O = E shifted left by one column
                act.copy(O[:, :, 0 : PW - 1], E[:, :, 1:PW]).then_inc(convA, 1)

        @block.vector
        def _(ve: bass.BassVectorEngine):
            ve.memset(acc[:, :, :], NEG)
            for S in (E0, E1, O0, O1):
                ve.memset(S[:, :, 0:R], NEG)
                ve.memset(S[:, :, R + W :], NEG)
            # -1e30 pad rows of the f32 slab (blocks 0 / 7 halo; engine
            # partition base must be 32-aligned, blocks 1/6 are re-DMAed)
            ve.memset(Sf[0:32, 0:R, :], NEG)
            ve.memset(Sf[96:128, HROWS - R : HROWS, :], NEG)
            ve.memset(acc[0:32, 0:1, 0:1], NEG).then_inc(initD, 1)
            for o in range(Or):
                ve.wait_ge(convA, o + 1)
                bands, _, center = sched[o]
                last = None
                if center is not None:
                    # cost at the center is exactly 0.0: plain max
                    last = ve.tensor_tensor(
                        out=acc[:, :, :],
                        in0=acc[:, :, :],
                        in1=eo_ap(o, 0, 0),
                        op=mybir.AluOpType.max,
                    )
                for band in bands:
                    cmid = float(np.float32((band[0][2] + band[-1][2]) / 2.0))
                    # max-tree over all band members into tmp
                    dy, dx, _ = band[0]
                    ve.tensor_tensor(
                        out=tmp[:, :, :],
                        in0=eo_ap(o, dy, dx),
                        in1=eo_ap(o, -dy, -dx),
                        op=mybir.AluOpType.max,
                    )
                    for dy, dx, _ in band[1:]:
                        for sy, sx in ((dy, dx), (-dy, -dx)):
                            ve.tensor_tensor(
                                out=tmp[:, :, :],
                                in0=tmp[:, :, :],
                                in1=eo_ap(o, sy, sx),
                                op=mybir.AluOpType.max,
                            )
                    ve.tensor_scalar_sub(tmp[:, :, :], tmp[:, :, :], cmid)
                    last = ve.tensor_tensor(
                        out=acc[:, :, :],
                        in0=acc[:, :, :],
                        in1=tmp[:, :, :],
                        op=mybir.AluOpType.max,
                    )
                last.then_inc(cmpD, 1)
            # merge GPSIMD accumulator and emit f32 into Sf's first 32 rows
            if HAS_GPS:
                ve.wait_ge(cmpG, Or)
                ve.tensor_tensor(
                    out=acc[:, :, :],
                    in0=acc[:, :, :],
                    in1=acc2[:, :, :],
                    op=mybir.AluOpType.max,
                )
            ve.tensor_copy(Sf[:, 0:BLK, :], acc[:, :, :]).then_inc(mrgD, 1)

        if HAS_GPS:

            @block.gpsimd
            def _(gp: bass.BassGpSimd):
                gp.memset(acc2[:, :, :], NEG)
                for o in range(Or):
                    gp.wait_ge(convA, o + 1)
                    _, gps_pairs, _ = sched[o]
                    last = None
                    for dy, dx, c in gps_pairs:
                        last = gp.tensor_tensor(
                            out=acc2[:, :, :],
                            in0=eo_ap(o, dy, dx),
                            in1=eo_ap(o, -dy, -dx),
                            op=mybir.AluOpType.max,
                        )
                    last.then_inc(cmpG, 1)

    return nc


_NC_CACHE = None


def _get_nc():
    global _NC_CACHE
    if _NC_CACHE is None:
        _NC_CACHE = _build_nc()
    return _NC_CACHE


def kernel(**inputs) -> np.ndarray:
    x = np.asarray(inputs["x"], dtype=np.float32)
    assert x.shape == (B, C, Or, H, W), x.shape
    nc = _get_nc()
    in_maps = [{"x": np.ascontiguousarray(x[i])} for i in range(B)]
    trace = bool(int(os.environ.get("BASS_KERNEL_TRACE", "0")))
    res = run_bass_kernel_spmd(nc, in_maps, core_ids=list(range(B)), trace=trace)
    if trace:
        kernel.last_exec_time_ns = res.exec_time_ns
        kernel.last_results = res
    out = np.stack([res.results[i]["out"] for i in range(B)], axis=0)
    return out.astype(np.float32, copy=False)
